# revision 42
# baseline (speedup 1.0000x reference)
"""ODE-RNN Trainium2 kernel (chain-latency optimized).

Math (matches jax reference within fp16 tolerance, rel ~6.8e-4):
  per step t (times from batch[0,:,0], shared across batch):
    hp = midpoint rule for dh/dt = tanh(h @ A) over [t_prev, t]
         (A = W1.T @ W2.T, biases zero)
    gru: r = sig(gr), w = m*(1-z) = sig(-gz - 40*(1-m))  [z weights
         pre-negated and the mask folded into the z pre-activation via
         an extra fold-matmul row, so one sigmoid emits w directly],
         n = tanh(gi_n + r*gh_n), h = hp + (w*n - w*hp)

Device layout: H on partitions ([128, KT, BL]), batch sharded 8 ways
(BL=32 rows/core), weights replicated, fp16 matmuls with fp32 PSUM.
PSUM tiles keep all matmul output offsets <= 32 elements (offset-64+
outputs crash the runtime), so r/z/n each get their own bank.

Critical chain per step (everything else runs off-chain; glue ops all on
DVE — real-HW GPSIMD is far slower than the cost model claims):
  tanh k1 -> stage2 matmul -> tanh k2 -> ks=dt*k2 (DVE) -> GRU tail
  matmul (r group stops first) -> sigmoid r -> tmpn=psn*r (DVE) ->
  argn=tmpn+gi (DVE) -> tanh n -> wn=w*n (DVE) -> A.T wn matmul -> k1.
The next step's U = A.T h is built as A.T hp + A.T wn - A.T wp with the
hp/wp parts accumulated off-chain (hp, wp=w*hp available before n), so
only the wn matmul group sits between tanh n and the next tanh k1.
The f32 state uses e = wn - wp, so h32 = hp32 + e costs two DVE ops.
"""
import numpy as np

import concourse.bass as bass
import concourse.bacc as bacc
import concourse.tile as tile
from concourse import mybir
from concourse.bass_utils import run_bass_kernel_spmd

B, T, H, D = 256, 64, 256, 512
NCORES = 8
BL = B // NCORES          # 32 batch rows per core
KT = H // 128             # 2 contraction tiles
F32 = mybir.dt.float32
F16 = mybir.dt.float16
AF = mybir.ActivationFunctionType
OP = mybir.AluOpType

STAGES = 2    # 2 = midpoint (~8e-4), 1 = Euler (~1.68e-2)
U_TRICK = True  # False: direct U = A.T h16 (longer chain, debug)
GATE_PRED = False  # gates from Euler predictor dt*k1 (k2 off chain)


def _build_program(dts, repeat=1):
    nc = bacc.Bacc(None, target_bir_lowering=False)

    a_d = nc.dram_tensor("a16", [128, KT * H], F16, kind="ExternalInput")
    an_d = nc.dram_tensor("an16", [128, KT * H], F16, kind="ExternalInput")
    whh_d = nc.dram_tensor("whh16", [128, KT, 3 * H], F16, kind="ExternalInput")
    a1_d = None
    if STAGES == 2:
        a1_d = nc.dram_tensor("a1s", [128, T, KT * H], F16, kind="ExternalInput")
    foldw_d = nc.dram_tensor("foldw", [96, 128], F16, kind="ExternalInput")
    foldx_d = nc.dram_tensor("foldx", [96, T, 2 * BL], F16, kind="ExternalInput")
    gi_d = nc.dram_tensor("gi_n", [T, 128, KT, BL], F32, kind="ExternalInput")
    out_d = nc.dram_tensor("h_out", [KT, 128, BL], F32, kind="ExternalOutput")

    with tile.TileContext(nc) as tc:
        with (
            tc.tile_pool(name="const", bufs=1) as const,
            tc.tile_pool(name="state", bufs=3) as state,
            tc.tile_pool(name="tmp", bufs=4) as tmp,
            tc.tile_pool(name="ps_stage", bufs=2, space="PSUM") as ps_stage,
            tc.tile_pool(name="ps_r", bufs=2, space="PSUM") as ps_r,
            tc.tile_pool(name="ps_z", bufs=2, space="PSUM") as ps_z,
            tc.tile_pool(name="ps_n", bufs=2, space="PSUM") as ps_n,
        ):
            # ---- preload constants ----
            a_sb = const.tile([128, KT * H], F16)
            nc.sync.dma_start(out=a_sb, in_=a_d[:, :])
            an_sb = const.tile([128, KT * H], F16)
            nc.sync.dma_start(out=an_sb, in_=an_d[:, :])
            a1_sb = None
            if STAGES == 2:
                a1_sb = const.tile([128, T, KT * H], F16)
                for t0 in range(0, T, 8):   # chunked: under 64KB/partition/desc
                    t1 = min(t0 + 8, T)
                    nc.sync.dma_start(out=a1_sb[:, t0:t1, :],
                                      in_=a1_d[:, t0:t1, :])
            whh_sb = const.tile([128, KT, 3 * H], F16)
            nc.sync.dma_start(out=whh_sb, in_=whh_d[:, :, :])
            foldw_sb = const.tile([96, 128], F16)
            nc.sync.dma_start(out=foldw_sb, in_=foldw_d[:, :])
            foldx_sb = const.tile([96, T, 2 * BL], F16)
            nc.sync.dma_start(out=foldx_sb, in_=foldx_d[:, :, :])
            gi_sb = const.tile([128, T, KT, BL], F32)
            for t in range(T):
                nc.sync.dma_start(out=gi_sb[:, t, :, :], in_=gi_d[t, :, :, :])

            def lhsT_of(sb, k, m):
                return sb[:, k * H + m * 128:k * H + (m + 1) * 128]

            def whh_lhsT(k, g):
                return whh_sb[:, k, g * 128:(g + 1) * 128]

            def mm(ps, lhsT, rhs, start=False, stop=False):
                nc.tensor.matmul(ps, lhsT, rhs, start=start, stop=stop,
                                 skip_group_check=True)

            def fold_mms(psr, psz, psn, t):
                # gi_r -> psr; -gi_z -> psz (rows pre-negated on host);
                # b_hh_n -> psn.  One start=True per tile (zeroes its bank).
                mm(psr[:, :, :], foldw_sb[0:10, :], foldx_sb[0:10, t, :],
                   start=True)
                mm(psz[:, :, :], foldw_sb[32:44, :], foldx_sb[32:44, t, :],
                   start=True)
                mm(psn[:, :, :], foldw_sb[64:68, :], foldx_sb[64:68, t, :],
                   start=True)

            def main_mms(psr, psz, psn, h16):
                for g in range(6):
                    dst = (psr, psr, psz, psz, psn, psn)[g][:, g % 2, :]
                    for k in range(KT):
                        mm(dst, whh_lhsT(k, g), h16[:, k, :])

            def body():
                h32 = state.tile([128, KT, BL], F32, tag="h32")
                nc.vector.memset(h32, 0.0)
                h16 = state.tile([128, KT, BL], F16, tag="h16")
                nc.vector.memset(h16, 0.0)

                # U_0 = A.T h_0 (= 0); gate pre-accumulation for t=0
                ps1 = ps_stage.tile([128, KT, BL], F32, tag="stage")
                for m in range(2):
                    for k in range(KT):
                        mm(ps1[:, m, :], lhsT_of(a_sb, k, m), h16[:, k, :],
                           start=(m == 0 and k == 0),
                           stop=(STAGES == 1 and m == 1 and k == KT - 1))
                psr = ps_r.tile([128, 2, BL], F32, tag="r")
                psz = ps_z.tile([128, 2, BL], F32, tag="z")
                psn = ps_n.tile([128, 2, BL], F32, tag="n")
                fold_mms(psr, psz, psn, 0)
                main_mms(psr, psz, psn, h16)

                for t in range(T):
                    dt = float(dts[t])

                    # ---- ODE stage 1 ----
                    k1h = tmp.tile([128, KT, BL], F16, tag="k1h")
                    nc.scalar.activation(k1h, ps1, AF.Tanh)
                    if GATE_PRED:
                        # gates consume the Euler predictor dt*k1 (O(dt^2)
                        # gate perturbation) so the k2 leg is off the chain
                        ksg = tmp.tile([128, KT, BL], F16, tag="ksg")
                        nc.vector.tensor_scalar_mul(ksg, k1h, dt)
                        for g in range(6):
                            dst = (psr, psr, psz, psz, psn, psn)[g][:, g % 2, :]
                            for k in range(KT):
                                mm(dst, whh_lhsT(k, g), ksg[:, k, :],
                                   stop=(g % 2 == 1 and k == KT - 1))
                        a1t = a1_sb[:, t, :]
                        for m in range(2):
                            for k in range(KT):
                                mm(ps1[:, m, :], lhsT_of(a1t, k, m),
                                   k1h[:, k, :], stop=(m == 1 and k == KT - 1))
                        r32 = tmp.tile([128, KT, BL], F32, tag="r32")
                        nc.scalar.activation(r32, psr, AF.Sigmoid)
                        w32 = tmp.tile([128, KT, BL], F32, tag="w32")
                        nc.scalar.activation(w32, psz, AF.Sigmoid)
                        tmpn = tmp.tile([128, KT, BL], F32, tag="tmpn")
                        nc.vector.tensor_mul(tmpn, psn, r32)
                        argn = tmp.tile([128, KT, BL], F32, tag="argn")
                        nc.vector.tensor_add(argn, tmpn, gi_sb[:, t, :, :])
                        k2h = tmp.tile([128, KT, BL], F16, tag="k2h")
                        nc.scalar.activation(k2h, ps1, AF.Tanh)
                        hp32 = state.tile([128, KT, BL], F32, tag="hp32")
                        nc.vector.scalar_tensor_tensor(hp32, k2h, dt, h32,
                                                       op0=OP.mult, op1=OP.add)
                        hp16 = state.tile([128, KT, BL], F16, tag="hp16")
                        nc.vector.tensor_copy(hp16, hp32)
                        wp = tmp.tile([128, KT, BL], F16, tag="wp")
                        nc.vector.tensor_mul(wp, w32, hp32)
                    else:
                        if STAGES == 2:
                            a1t = a1_sb[:, t, :]
                            for m in range(2):
                                for k in range(KT):
                                    mm(ps1[:, m, :], lhsT_of(a1t, k, m),
                                       k1h[:, k, :],
                                       stop=(m == 1 and k == KT - 1))
                            k2h = tmp.tile([128, KT, BL], F16, tag="k2h")
                            nc.scalar.activation(k2h, ps1, AF.Tanh)
                        else:
                            k2h = k1h
                        ksg = tmp.tile([128, KT, BL], F16, tag="ksg")
                        nc.vector.tensor_scalar_mul(ksg, k2h, dt)
                        hp32 = state.tile([128, KT, BL], F32, tag="hp32")
                        nc.vector.scalar_tensor_tensor(hp32, k2h, dt, h32,
                                                       op0=OP.mult, op1=OP.add)
                        hp16 = state.tile([128, KT, BL], F16, tag="hp16")
                        nc.vector.tensor_copy(hp16, hp32)
                        for g in range(6):
                            dst = (psr, psr, psz, psz, psn, psn)[g][:, g % 2, :]
                            for k in range(KT):
                                mm(dst, whh_lhsT(k, g), ksg[:, k, :],
                                   stop=(g % 2 == 1 and k == KT - 1))
                        r32 = tmp.tile([128, KT, BL], F32, tag="r32")
                        nc.scalar.activation(r32, psr, AF.Sigmoid)
                        w32 = tmp.tile([128, KT, BL], F32, tag="w32")
                        nc.scalar.activation(w32, psz, AF.Sigmoid)
                        tmpn = tmp.tile([128, KT, BL], F32, tag="tmpn")
                        nc.vector.tensor_mul(tmpn, psn, r32)
                        argn = tmp.tile([128, KT, BL], F32, tag="argn")
                        nc.vector.tensor_add(argn, tmpn, gi_sb[:, t, :, :])
                        wp = tmp.tile([128, KT, BL], F16, tag="wp")
                        nc.vector.tensor_mul(wp, w32, hp32)

                    n32 = tmp.tile([128, KT, BL], F32, tag="n32")
                    nc.scalar.activation(n32, argn, AF.Tanh)
                    wn = tmp.tile([128, KT, BL], F16, tag="wn")
                    nc.vector.tensor_mul(wn, w32, n32)

                    # ---- next-step U = A.T hp + A.T wn - A.T wp ----
                    if t + 1 < T and U_TRICK:
                        ps1n = ps_stage.tile([128, KT, BL], F32, tag="stage")
                        for m in range(2):
                            for k in range(KT):
                                mm(ps1n[:, m, :], lhsT_of(a_sb, k, m),
                                   hp16[:, k, :], start=(m == 0 and k == 0))
                        for m in range(2):
                            for k in range(KT):
                                mm(ps1n[:, m, :], lhsT_of(an_sb, k, m),
                                   wp[:, k, :])
                        for m in range(2):
                            for k in range(KT):
                                mm(ps1n[:, m, :], lhsT_of(a_sb, k, m),
                                   wn[:, k, :],
                                   stop=(STAGES == 1 and m == 1 and k == KT - 1))
                        ps1 = ps1n

                    # ---- state update: e = wn - wp (fp16 diff, off-chain)
                    e32 = tmp.tile([128, KT, BL], F32, tag="e32")
                    nc.vector.tensor_sub(e32, wn, wp)
                    h32 = state.tile([128, KT, BL], F32, tag="h32")
                    nc.vector.tensor_add(h32, hp32, e32)
                    h16 = state.tile([128, KT, BL], F16, tag="h16")
                    nc.vector.tensor_copy(h16, h32)

                    if t + 1 < T and not U_TRICK:
                        ps1n = ps_stage.tile([128, KT, BL], F32, tag="stage")
                        for m in range(2):
                            for k in range(KT):
                                mm(ps1n[:, m, :], lhsT_of(a_sb, k, m),
                                   h16[:, k, :], start=(m == 0 and k == 0),
                                   stop=(STAGES == 1 and m == 1 and k == KT - 1))
                        ps1 = ps1n

                    # ---- next-step gate pre-accumulation ----
                    if t + 1 < T:
                        psr_n = ps_r.tile([128, 2, BL], F32, tag="r")
                        psz_n = ps_z.tile([128, 2, BL], F32, tag="z")
                        psn_n = ps_n.tile([128, 2, BL], F32, tag="n")
                        fold_mms(psr_n, psz_n, psn_n, t + 1)
                        main_mms(psr_n, psz_n, psn_n, h16)
                        psr, psz, psn = psr_n, psz_n, psn_n

                return h32

            if repeat == 1:
                hfin = body()
            else:
                with tc.For_i(0, repeat, 1):
                    hfin = body()

            for k in range(KT):
                nc.sync.dma_start(out=out_d[k, :, :], in_=hfin[:, k, :])

    nc.finalize()
    return nc


def _prepare_inputs(batch, mask, W1, b1, W2, b2, W_ih, b_ih, W_hh, b_hh):
    batch = np.asarray(batch, np.float32)
    mask = np.asarray(mask, np.float32)
    W1 = np.asarray(W1, np.float32); b1 = np.asarray(b1, np.float32)
    W2 = np.asarray(W2, np.float32); b2 = np.asarray(b2, np.float32)
    W_ih = np.asarray(W_ih, np.float32); b_ih = np.asarray(b_ih, np.float32)
    W_hh = np.asarray(W_hh, np.float32); b_hh = np.asarray(b_hh, np.float32)

    A = (W1.T.astype(np.float64) @ W2.T.astype(np.float64)).astype(np.float32)
    c = (b1.astype(np.float64) @ W2.T.astype(np.float64) + b2).astype(np.float32)
    assert np.abs(c).max() == 0.0, "nonzero ODE bias not wired into ACT bias"

    times = batch[0, :, 0].astype(np.float64)
    dts = np.diff(np.concatenate([[0.0], times]))

    def a_blocks(M, dtype=np.float16):   # [H, H] -> [128, KT*H] k-tile concat
        return np.ascontiguousarray(np.concatenate(
            [M[k * 128:(k + 1) * 128, :] for k in range(KT)], axis=1)).astype(dtype)

    a16 = a_blocks(A)
    an16 = a_blocks(-A)
    a1s = None
    if STAGES == 2:
        a1s = np.ascontiguousarray(np.stack(
            [a_blocks((A.astype(np.float64) * (d / 2)).astype(np.float32))
             for d in dts]).transpose(1, 0, 2))           # [128,T,KT*H] fp16

    # W_hh.T with z-gate columns negated (so sigmoid gives 1-z directly)
    WhhT = np.ascontiguousarray(W_hh.T).astype(np.float64)
    WhhT[:, 2 * 128:4 * 128] *= -1.0
    WhhT = WhhT.astype(np.float32)
    whh16 = np.ascontiguousarray(
        np.stack([WhhT[k * 128:(k + 1) * 128, :] for k in range(KT)], axis=1)
    ).astype(np.float16)

    # fold weights: exact fp16 split of W_ih and (b_ih+b_hh) per gate half.
    # z rows negated. n rows carry b_hh_n only (gi_n precomputed separately).
    bsum = b_ih + b_hh
    foldw = np.zeros((96, 128), np.float16)
    for reg in range(4):                                 # r0 r1 z0 z1
        sgn = 1.0 if reg < 2 else -1.0
        wslice = sgn * W_ih[reg * 128:(reg + 1) * 128, 0]
        whi = wslice.astype(np.float16)
        wlo = (wslice - whi.astype(np.float32)).astype(np.float16)
        bs = sgn * bsum[reg * 128:(reg + 1) * 128]
        bshi = bs.astype(np.float16)
        bslo = (bs - bshi.astype(np.float32)).astype(np.float16)
        if reg < 2:
            base = reg * 5                               # r: 0/5
        else:
            base = 32 + (reg - 2) * 6                    # z: 32/38 (6 rows:
        foldw[base + 0] = whi                            #  5 + mask row)
        foldw[base + 1] = wlo
        foldw[base + 2] = whi
        foldw[base + 3] = bshi
        foldw[base + 4] = bslo
        if reg >= 2:
            foldw[base + 5] = 1.0                        # mask row
    for reg in range(2):                                 # n0 n1 (b_hh only)
        bn = b_hh[2 * H + reg * 128:2 * H + (reg + 1) * 128]
        bnhi = bn.astype(np.float16)
        bnlo = (bn - bnhi.astype(np.float32)).astype(np.float16)
        foldw[64 + reg * 2 + 0] = bnhi
        foldw[64 + reg * 2 + 1] = bnlo

    xs = batch[:, :, 1]
    gi_n_full = (xs[:, :, None] * W_ih[None, None, 2 * H:, 0]
                 + b_ih[None, None, 2 * H:]).astype(np.float32)  # [B,T,H]

    in_maps = []
    for ci in range(NCORES):
        bs = slice(ci * BL, (ci + 1) * BL)
        xs_c = xs[bs].T                                  # [T, BL]
        xhi = xs_c.astype(np.float16)
        xlo = (xs_c - xhi.astype(np.float32)).astype(np.float16)
        foldx = np.zeros((96, T, 2 * BL), np.float16)
        mneg = (-40.0 * (1.0 - mask[bs].T)).astype(np.float16)   # [T, BL]
        for reg01, sl in ((0, slice(0, BL)), (1, slice(BL, 2 * BL))):
            for zbase, step in ((0, 5), (32, 6)):        # r rows, z rows
                base = zbase + reg01 * step
                foldx[base + 0, :, sl] = xhi
                foldx[base + 1, :, sl] = xhi
                foldx[base + 2, :, sl] = xlo
                foldx[base + 3, :, sl] = 1.0
                foldx[base + 4, :, sl] = 1.0
                if step == 6:
                    foldx[base + 5, :, sl] = mneg        # -40*(1-m): m*zc
            foldx[64 + reg01 * 2 + 0, :, sl] = 1.0       # n ones
            foldx[64 + reg01 * 2 + 1, :, sl] = 1.0
        gi_c = gi_n_full[bs].transpose(1, 2, 0)          # [T, H, BL]
        gi_c = np.ascontiguousarray(
            gi_c.reshape(T, KT, 128, BL).transpose(0, 2, 1, 3))
        im = {
            "a16": a16, "an16": an16, "whh16": whh16,
            "foldw": foldw, "foldx": np.ascontiguousarray(foldx),
            "gi_n": gi_c,
        }
        if STAGES == 2:
            im["a1s"] = a1s
        in_maps.append(im)
    return dts, in_maps


def kernel(batch, mask, W1, b1, W2, b2, W_ih, b_ih, W_hh, b_hh):
    dts, in_maps = _prepare_inputs(batch, mask, W1, b1, W2, b2,
                                   W_ih, b_ih, W_hh, b_hh)
    nc = _build_program([float(d) for d in dts])
    res = run_bass_kernel_spmd(nc, in_maps, core_ids=list(range(NCORES)))

    out = np.empty((B, H), np.float32)
    for ci in range(NCORES):
        ho = res.results[ci]["h_out"]                    # [KT, 128, BL]
        for k in range(KT):
            out[ci * BL:(ci + 1) * BL, k * 128:(k + 1) * 128] = ho[k].T
    return out
